# revision 28
# baseline (speedup 1.0000x reference)
"""Trainium2 Bass kernel: DepthScalingLayer3D (masked-bbox depth rescale).

Reference semantics:
  mask = sparse_depth_masks[0,0] != 0
  pts(d) = [(x-cx)*d/fx, (y-cy)*d/fy, d]  per pixel
  scale(d) = || masked_max(pts) - masked_min(pts) ||
  out = (scale(sparse)/scale(dense)) * depth_estimations, std = 1.0

Key decomposition: x3 = colscale[j]*d and y3 = rowscale[i]*d with scales
constant per column/row, and fp multiplication by a constant is monotone.
So the global masked min/max of x3/y3/z3 all derive from per-column and
per-row masked min/max of d alone.  Per core (256 rows): two fused
clamp+row-reduce passes per depth tensor (tensor_tensor_reduce), a 32x32
stream-transpose + reduce for per-column extremes, tiny finalize ops,
a 12-float AllGather across the 8 cores, then one scale pass over the
dense shard.

Sharding: rows (H) split 8 ways, 256 rows/core.  kernel() takes the full
inputs, shards on host, runs the SPMD NEFF on cores 0-7, gathers shards.
"""

import os
import sys

import numpy as np

sys.path.insert(0, "/opt/trn_rl_repo")

from concourse import bacc, mybir, tile  # noqa: E402
import concourse.bass_isa as bass_isa  # noqa: E402
from concourse.bass_utils import run_bass_kernel_spmd  # noqa: E402

F32 = mybir.dt.float32
Alu = mybir.AluOpType
Act = mybir.ActivationFunctionType
AxX = mybir.AxisListType.X

P = 128          # SBUF partitions
W = 2048         # image width
H = 2048         # image height
NCORES = 8
RPC = H // NCORES  # rows per core (256)
NT = RPC // P      # row tiles per core (2)
BIG = 1.0e18       # mask sentinel; |d| < 64 so d +- BIG rounds to +-BIG exactly
NEGINF = -3.0e38
POSINF = 3.0e38

# stat columns in the packed [128, 12] partials tile:
#   0..5  = maxes  (sparse x, y, z, dense x, y, z)
#   6..11 = mins (stored negated after finalize) in the same order
_cache = {}
DEBUG = bool(os.environ.get("KERNEL_DEBUG"))


def _register_clamp_ops():
    """Fused (d +- neg) clamp with max/min free-axis accumulate, as custom
    DVE ops (the native TENSOR_TENSOR_REDUCE opcode faults on this runtime).
    Returns (hi_op, lo_op)."""
    from concourse import dve_ops
    from concourse.dve_spec import Spec, Src0, Src1, C0, maxx, minn, lower
    from concourse.dve_spec import _has_src1 as has_src1
    from concourse.dve_uop import DveOpSpec

    existing = {op.name: op for op in dve_ops.OPS}
    if "ANT_CLAMP_HI_MAX" in existing:
        return existing["ANT_CLAMP_HI_MAX"], existing["ANT_CLAMP_LO_MIN"]

    hi = dve_ops.DveOp("ANT_CLAMP_HI_MAX",
                       Spec(body=Src0 + Src1, accum=maxx, accum_init=C0),
                       subdim=False, uops_sha={})
    lo = dve_ops.DveOp("ANT_CLAMP_LO_MIN",
                       Spec(body=Src0 - Src1, accum=minn, accum_init=C0),
                       subdim=False, uops_sha={})
    row = max(dve_ops._SUB_OPCODE_FOR_NAME.values()) + 1
    for op in (hi, lo):
        dve_ops.OPS.append(op)
        dve_ops.CUSTOM_DVE_SPECS[op.name] = op.spec
        dve_ops._SUB_OPCODE_FOR_NAME[op.name] = row
        row += 1
        for ver in ("v3", "v4"):
            try:
                spec = DveOpSpec(
                    name=op.name,
                    opcode=dve_ops.get_dve_sub_opcode(op.name),
                    uops=lower(op.spec, ver=ver),
                    rd1_en=has_src1(op.spec))
                op.uops_sha[ver] = spec.sha(ver)
            except Exception:
                pass
    return hi, lo


def _build_nc():
    hi_op, lo_op = _register_clamp_ops()
    nc = bacc.Bacc("TRN2", target_bir_lowering=False, debug=False,
                   num_devices=NCORES)

    dse = nc.dram_tensor("d_sparse", [RPC, W], F32, kind="ExternalInput").ap()
    dde = nc.dram_tensor("d_dense", [RPC, W], F32, kind="ExternalInput").ap()
    mke = nc.dram_tensor("mask", [RPC, W], F32, kind="ExternalInput").ap()
    cse = nc.dram_tensor("cscale", [P, 64], F32, kind="ExternalInput").ap()
    rse = nc.dram_tensor("rscale", [P, NT], F32, kind="ExternalInput").ap()
    oute = nc.dram_tensor("out", [RPC, W], F32, kind="ExternalOutput").ap()
    if DEBUG:
        dbge = nc.dram_tensor("dbg", [1, 32], F32,
                              kind="ExternalOutput").ap()

    with tile.TileContext(nc) as tc:
        with tc.tile_pool(name="big", bufs=1) as bp, \
             tc.tile_pool(name="small", bufs=1) as sp, \
             tc.tile_pool(name="dram", bufs=1, space="DRAM") as dp:

            mk = [bp.tile([P, W], F32, tag=f"mk{t}", name=f"mk{t}")
                  for t in range(NT)]
            neg = [bp.tile([P, W], F32, tag=f"neg{t}", name=f"neg{t}")
                   for t in range(NT)]
            ds = [bp.tile([P, W], F32, tag=f"ds{t}", name=f"ds{t}")
                  for t in range(NT)]
            dd = [bp.tile([P, W], F32, tag=f"dd{t}", name=f"dd{t}")
                  for t in range(NT)]

            for t in range(NT):
                nc.sync.dma_start(out=mk[t][:], in_=mke[t * P:(t + 1) * P, :])
                # neg = (m * BIG) - BIG  ->  0 at masked, -BIG at unmasked
                # (on ScalarE: out = Copy(in*scale + bias); keeps DVE free)
                nc.scalar.activation(neg[t][:], mk[t][:], Act.Copy,
                                     bias=-BIG, scale=BIG)
                nc.sync.dma_start(out=ds[t][:], in_=dse[t * P:(t + 1) * P, :])
            for t in range(NT):
                nc.sync.dma_start(out=dd[t][:], in_=dde[t * P:(t + 1) * P, :])

            csc = sp.tile([P, 64], F32, tag="csc", name="csc")
            rsc = sp.tile([P, NT], F32, tag="rsc", name="rsc")
            # gpsimd (SWDGE) queue: the dynamic-HW DMA queue's completion
            # semaphore is miscounted for these small strided copies on this
            # runtime, letting consumers race ahead of the data.
            nc.gpsimd.dma_start(out=csc[:], in_=cse[:, :])
            nc.gpsimd.dma_start(out=rsc[:], in_=rse[:, :])


            def bounds(dtiles, nm):
                """Compute masked bbox partials of tensor `dtiles`, reduce
                across partitions, and kick off this tensor's AllGather.
                Returns the [8, 6] gathered stats dram tile."""
                ppack = sp.tile([P, 6], F32, tag=f"ppack{nm}",
                                name=f"ppack{nm}")
                nc.vector.memset(ppack[:, 0:3], NEGINF)
                nc.vector.memset(ppack[:, 3:6], POSINF)
                hi = [bp.tile([P, W], F32, tag=f"hi{nm}{t}", name=f"hi{nm}{t}")
                      for t in range(NT)]
                lo = [bp.tile([P, W], F32, tag=f"lo{nm}{t}", name=f"lo{nm}{t}")
                      for t in range(NT)]
                rowhi = sp.tile([P, NT], F32, tag=f"rowhi{nm}", name=f"rowhi{nm}")
                rowlo = sp.tile([P, NT], F32, tag=f"rowlo{nm}", name=f"rowlo{nm}")
                for t in range(NT):
                    # hi = d + neg (d at masked, -BIG at unmasked); row max
                    nc.vector._custom_dve(
                        hi_op, out=hi[t][:], in0=dtiles[t][:], in1=neg[t][:],
                        s0=NEGINF, accum_out=rowhi[:, t:t + 1])
                    # lo = d - neg (d at masked, +BIG at unmasked); row min
                    nc.vector._custom_dve(
                        lo_op, out=lo[t][:], in0=dtiles[t][:], in1=neg[t][:],
                        s0=POSINF, accum_out=rowlo[:, t:t + 1])
                # combine the two row tiles
                nc.vector.tensor_tensor(hi[0][:], hi[0][:], hi[1][:], Alu.max)
                nc.vector.tensor_tensor(lo[0][:], lo[0][:], lo[1][:], Alu.min)

                # per-column extremes: fused 32x32 block transpose + in-block
                # row reduce in one TensorReduce(apply_transpose) pass.
                colhi = sp.tile([P, 64], F32, tag=f"colhi{nm}", name=f"colhi{nm}")
                collo = sp.tile([P, 64], F32, tag=f"collo{nm}", name=f"collo{nm}")
                nc.vector.tensor_reduce(
                    colhi[:], hi[0][:].rearrange("p (a b) -> p a b", b=32),
                    AxX, Alu.max, apply_transpose=True)
                nc.vector.tensor_reduce(
                    collo[:], lo[0][:].rearrange("p (a b) -> p a b", b=32),
                    AxX, Alu.min, apply_transpose=True)
                # colhi[32*pg+a, fg] = extreme of column 32*fg+a over the
                # pg-th 32-row group (of rows folded mod 128 already).  The
                # partition-group fold happens for free in the final
                # partition_all_reduce, so finalize per group directly with
                # cscale replicated per group.

                # x bounds: candidates cscale * colext (both extremes; sign)
                pa = sp.tile([P, 64], F32, tag=f"pa{nm}", name=f"pa{nm}")
                pb = sp.tile([P, 64], F32, tag=f"pb{nm}", name=f"pb{nm}")
                ph = sp.tile([P, 64], F32, tag=f"ph{nm}", name=f"ph{nm}")
                pl = sp.tile([P, 64], F32, tag=f"pl{nm}", name=f"pl{nm}")
                nc.vector.tensor_tensor(pa[:], csc[:], colhi[:], Alu.mult)
                nc.vector.tensor_tensor(pb[:], csc[:], collo[:], Alu.mult)
                nc.vector.tensor_tensor(ph[:], pa[:], pb[:], Alu.max)
                nc.vector.tensor_tensor(pl[:], pa[:], pb[:], Alu.min)
                # (group, column) cells with no masked pixel hold the raw
                # sentinel (colhi == -BIG); their candidates are garbage, so
                # push them to -+3e38 where the max/min reduce ignores them.
                pen = sp.tile([P, 64], F32, tag=f"pen{nm}", name=f"pen{nm}")
                nc.vector.tensor_scalar(pen[:], colhi[:], -BIG, NEGINF,
                                        Alu.is_equal, Alu.mult)
                nc.vector.tensor_tensor(ph[:], ph[:], pen[:], Alu.add)
                nc.vector.tensor_tensor(pl[:], pl[:], pen[:], Alu.subtract)
                nc.vector.tensor_reduce(ppack[:, 0:1], ph[:], AxX, Alu.max)
                nc.vector.tensor_reduce(ppack[:, 3:4], pl[:], AxX, Alu.min)

                # y bounds: candidates rscale * rowext
                qa = sp.tile([P, NT], F32, tag=f"qa{nm}", name=f"qa{nm}")
                qb = sp.tile([P, NT], F32, tag=f"qb{nm}", name=f"qb{nm}")
                qh = sp.tile([P, NT], F32, tag=f"qh{nm}", name=f"qh{nm}")
                ql = sp.tile([P, NT], F32, tag=f"ql{nm}", name=f"ql{nm}")
                nc.vector.tensor_tensor(qa[:], rsc[:], rowhi[:], Alu.mult)
                nc.vector.tensor_tensor(qb[:], rsc[:], rowlo[:], Alu.mult)
                nc.vector.tensor_tensor(qh[:], qa[:], qb[:], Alu.max)
                nc.vector.tensor_tensor(ql[:], qa[:], qb[:], Alu.min)
                # same sentinel-proofing for fully-unmasked rows
                rpen = sp.tile([P, NT], F32, tag=f"rpen{nm}", name=f"rpen{nm}")
                nc.vector.tensor_scalar(rpen[:], rowhi[:], -BIG, NEGINF,
                                        Alu.is_equal, Alu.mult)
                nc.vector.tensor_tensor(qh[:], qh[:], rpen[:], Alu.add)
                nc.vector.tensor_tensor(ql[:], ql[:], rpen[:], Alu.subtract)
                nc.vector.tensor_reduce(ppack[:, 1:2], qh[:], AxX, Alu.max)
                nc.vector.tensor_reduce(ppack[:, 4:5], ql[:], AxX, Alu.min)

                # z bounds: plain masked extremes of d
                nc.vector.tensor_reduce(ppack[:, 2:3], rowhi[:],
                                        AxX, Alu.max)
                nc.vector.tensor_reduce(ppack[:, 5:6], rowlo[:],
                                        AxX, Alu.min)

                # negate mins so one max-allreduce covers everything, fold
                # partitions, and AllGather this tensor's 6 stats now --
                # the sparse collective's cross-core skew hides under the
                # dense tensor's compute.
                nc.vector.tensor_scalar(ppack[:, 3:6], ppack[:, 3:6], -1.0,
                                        None, Alu.mult)
                rall = sp.tile([P, 6], F32, tag=f"rall{nm}", name=f"rall{nm}")
                nc.gpsimd.partition_all_reduce(rall[:], ppack[:], channels=P,
                                               reduce_op=bass_isa.ReduceOp.max)
                cc_in = dp.tile([1, 6], F32, tag=f"cc_in{nm}",
                                name=f"cc_in{nm}")
                cc_out = dp.tile([NCORES, 6], F32, tag=f"cc_out{nm}",
                                 name=f"cc_out{nm}", addr_space="Shared")
                nc.gpsimd.dma_start(out=cc_in[:, :], in_=rall[0:1, :])
                nc.gpsimd.collective_compute(
                    "AllGather", Alu.bypass,
                    replica_groups=[list(range(NCORES))],
                    ins=[cc_in.opt()], outs=[cc_out.opt()])
                return cc_out

            cc_out_s = bounds(ds, "s")
            cc_out_d = bounds(dd, "d")

            # gather both tensors' stats and reduce across cores
            g1 = sp.tile([1, 2 * NCORES * 6], F32, tag="g1", name="g1")
            nc.gpsimd.dma_start(out=g1[0:1, 0:48], in_=cc_out_s[:, :])
            nc.gpsimd.dma_start(out=g1[0:1, 48:96], in_=cc_out_d[:, :])
            # gm layout: [smax xyz, snegmin xyz, dmax xyz, dnegmin xyz]
            gm = sp.tile([1, 12], F32, tag="gm", name="gm")
            nc.vector.tensor_reduce(
                gm[0:1, 0:6],
                g1[0:1, 0:48].rearrange("p (c s) -> p s c", c=NCORES),
                AxX, Alu.max)
            nc.vector.tensor_reduce(
                gm[0:1, 6:12],
                g1[0:1, 48:96].rearrange("p (c s) -> p s c", c=NCORES),
                AxX, Alu.max)

            # ratio = ||bbox_s|| / ||bbox_d||; diff = max + (-min), packed
            # [s xyz, d xyz] via one 3D-strided add
            diff = sp.tile([1, 6], F32, tag="diff", name="diff")
            gm3 = gm[0:1, :].rearrange("p (t h c) -> p t h c", t=2, h=2)
            nc.vector.tensor_tensor(
                diff[0:1, :].rearrange("p (t c) -> p t c", t=2),
                gm3[:, :, 0, :], gm3[:, :, 1, :], Alu.add)
            sq = sp.tile([1, 6], F32, tag="sq", name="sq")
            nc.vector.tensor_tensor(sq[0:1, :], diff[0:1, :], diff[0:1, :],
                                    Alu.mult)
            ssum = sp.tile([1, 2], F32, tag="ssum", name="ssum")
            nc.vector.tensor_reduce(
                ssum[0:1, :], sq[0:1, :].rearrange("p (a b) -> p a b", b=3),
                AxX, Alu.add)
            # sqrt via ScalarE LUT (~5e-4 rel) + one Newton step (~1e-7):
            # y1 = 0.5 * (y + x/y)
            sqr = sp.tile([1, 2], F32, tag="sqr", name="sqr")
            zbias = sp.tile([1, 1], F32, tag="zbias", name="zbias")
            nc.vector.memset(zbias[0:1, :], 0.0)
            nc.scalar.activation(sqr[0:1, :], ssum[0:1, :], Act.Sqrt,
                                 bias=zbias[0:1, 0:1])
            yrec = sp.tile([1, 2], F32, tag="yrec", name="yrec")
            nc.vector.reciprocal(yrec[0:1, :], sqr[0:1, :])
            xy = sp.tile([1, 2], F32, tag="xy", name="xy")
            nc.vector.tensor_tensor(xy[0:1, :], ssum[0:1, :], yrec[0:1, :],
                                    Alu.mult)
            nc.vector.tensor_tensor(xy[0:1, :], xy[0:1, :], sqr[0:1, :],
                                    Alu.add)
            nc.vector.tensor_scalar(xy[0:1, :], xy[0:1, :], 0.5, None,
                                    Alu.mult)
            rec = sp.tile([1, 1], F32, tag="rec", name="rec")
            nc.vector.reciprocal(rec[0:1, :], xy[0:1, 1:2])
            ratio = sp.tile([1, 1], F32, tag="ratio", name="ratio")
            nc.vector.tensor_tensor(ratio[0:1, :], xy[0:1, 0:1], rec[0:1, :],
                                    Alu.mult)
            rb = sp.tile([P, 1], F32, tag="rb", name="rb")
            nc.gpsimd.partition_broadcast(rb[:], ratio[0:1, :])

            if DEBUG:
                dbg = sp.tile([1, 32], F32, tag="dbg", name="dbg")
                nc.vector.memset(dbg[0:1, :], 0.0)
                nc.vector.tensor_copy(dbg[0:1, 0:12], gm[0:1, :])
                nc.vector.tensor_copy(dbg[0:1, 12:14], ssum[0:1, :])
                nc.vector.tensor_copy(dbg[0:1, 14:15], ratio[0:1, :])
                nc.sync.dma_start(out=dbge[:, :], in_=dbg[0:1, :])

            # scale the dense shard and store (one tile on ScalarE, one on
            # DVE -- both engines are idle in the tail)
            nc.scalar.mul(dd[0][:], dd[0][:], mul=rb[:, 0:1])
            nc.sync.dma_start(out=oute[0:P, :], in_=dd[0][:])
            nc.vector.tensor_scalar(dd[1][:], dd[1][:], rb[:, 0:1], None,
                                    Alu.mult)
            nc.sync.dma_start(out=oute[P:2 * P, :], in_=dd[1][:])

    nc.compile()
    return nc


LAST_RESULTS = None


def kernel(depth_estimations, sparse_depths, sparse_depth_masks, intrinsics):
    global LAST_RESULTS
    d = np.ascontiguousarray(
        np.asarray(depth_estimations, dtype=np.float32).reshape(H, W))
    s = np.ascontiguousarray(
        np.asarray(sparse_depths, dtype=np.float32).reshape(H, W))
    m = (np.asarray(sparse_depth_masks).reshape(H, W) != 0).astype(np.float32)
    fx, fy, cx, cy = [np.float32(v) for v in np.asarray(intrinsics).ravel()]

    # column scale (x-cx)/fx in transposed-block layout, replicated per
    # partition group: cscT[32*pg + a, fg] = c[32*fg + a]
    c = (np.arange(W, dtype=np.float32) - cx) / fx
    cscT = np.ascontiguousarray(
        np.tile(c.reshape(W // 32, 32).T, (4, 1)))  # [128, 64]

    if "nc" not in _cache:
        _cache["nc"] = _build_nc()
    nc = _cache["nc"]

    in_maps = []
    for i in range(NCORES):
        r0 = i * RPC
        r = (np.arange(r0, r0 + RPC, dtype=np.float32) - cy) / fy
        rscT = np.ascontiguousarray(r.reshape(NT, P).T)  # [128, NT]
        in_maps.append({
            "d_sparse": np.ascontiguousarray(s[r0:r0 + RPC]),
            "d_dense": np.ascontiguousarray(d[r0:r0 + RPC]),
            "mask": np.ascontiguousarray(m[r0:r0 + RPC]),
            "cscale": cscT,
            "rscale": rscT,
        })

    res = run_bass_kernel_spmd(nc, in_maps, list(range(NCORES)),
                               trace=bool(os.environ.get("BASS_TRACE")))
    LAST_RESULTS = res
    out = np.concatenate([res.results[i]["out"] for i in range(NCORES)],
                         axis=0).reshape(1, 1, H, W)
    std = np.float32(1.0)
    return out, std


# revision 31
# speedup vs baseline: 1.4766x; 1.4766x over previous
"""Trainium2 Bass kernel: DepthScalingLayer3D (masked-bbox depth rescale).

Reference semantics:
  mask = sparse_depth_masks[0,0] != 0
  pts(d) = [(x-cx)*d/fx, (y-cy)*d/fy, d]  per pixel
  scale(d) = || masked_max(pts) - masked_min(pts) ||
  out = (scale(sparse)/scale(dense)) * depth_estimations, std = 1.0

Key decomposition: x3 = colscale[j]*d and y3 = rowscale[i]*d with scales
constant per column/row, and fp multiplication by a constant is monotone.
So the global masked min/max of x3/y3/z3 all derive from per-column and
per-row masked min/max of d alone.  Per core (256 rows): two fused
clamp+row-reduce passes per depth tensor (tensor_tensor_reduce), a 32x32
stream-transpose + reduce for per-column extremes, tiny finalize ops,
a 12-float AllGather across the 8 cores, then one scale pass over the
dense shard.

Sharding: rows (H) split 8 ways, 256 rows/core.  kernel() takes the full
inputs, shards on host, runs the SPMD NEFF on cores 0-7, gathers shards.
"""

import os
import sys

import numpy as np

sys.path.insert(0, "/opt/trn_rl_repo")

from concourse import bacc, mybir, tile  # noqa: E402
import concourse.bass_isa as bass_isa  # noqa: E402
from concourse.bass_utils import run_bass_kernel_spmd  # noqa: E402

F32 = mybir.dt.float32
Alu = mybir.AluOpType
Act = mybir.ActivationFunctionType
AxX = mybir.AxisListType.X

P = 128          # SBUF partitions
W = 2048         # image width
H = 2048         # image height
NCORES = 8
RPC = H // NCORES  # rows per core (256)
NT = RPC // P      # row tiles per core (2)
BIG = 1.0e18       # mask sentinel; |d| < 64 so d +- BIG rounds to +-BIG exactly
NEGINF = -3.0e38
POSINF = 3.0e38

# stat columns in the packed [128, 12] partials tile:
#   0..5  = maxes  (sparse x, y, z, dense x, y, z)
#   6..11 = mins (stored negated after finalize) in the same order
_cache = {}
DEBUG = bool(os.environ.get("KERNEL_DEBUG"))


def _register_clamp_ops():
    """Fused (d +- neg) clamp with max/min free-axis accumulate, as custom
    DVE ops (the native TENSOR_TENSOR_REDUCE opcode faults on this runtime).
    Returns (hi_op, lo_op)."""
    from concourse import dve_ops
    from concourse.dve_spec import Spec, Src0, Src1, C0, maxx, minn, lower
    from concourse.dve_spec import _has_src1 as has_src1
    from concourse.dve_uop import DveOpSpec

    existing = {op.name: op for op in dve_ops.OPS}
    if "ANT_CLAMP_HI_MAX" in existing:
        return existing["ANT_CLAMP_HI_MAX"], existing["ANT_CLAMP_LO_MIN"]

    hi = dve_ops.DveOp("ANT_CLAMP_HI_MAX",
                       Spec(body=Src0 + Src1, accum=maxx, accum_init=C0),
                       subdim=False, uops_sha={})
    lo = dve_ops.DveOp("ANT_CLAMP_LO_MIN",
                       Spec(body=Src0 - Src1, accum=minn, accum_init=C0),
                       subdim=False, uops_sha={})
    row = max(dve_ops._SUB_OPCODE_FOR_NAME.values()) + 1
    for op in (hi, lo):
        dve_ops.OPS.append(op)
        dve_ops.CUSTOM_DVE_SPECS[op.name] = op.spec
        dve_ops._SUB_OPCODE_FOR_NAME[op.name] = row
        row += 1
        for ver in ("v3", "v4"):
            try:
                spec = DveOpSpec(
                    name=op.name,
                    opcode=dve_ops.get_dve_sub_opcode(op.name),
                    uops=lower(op.spec, ver=ver),
                    rd1_en=has_src1(op.spec))
                op.uops_sha[ver] = spec.sha(ver)
            except Exception:
                pass
    return hi, lo


def _build_nc():
    hi_op, lo_op = _register_clamp_ops()
    nc = bacc.Bacc("TRN2", target_bir_lowering=False, debug=False,
                   num_devices=NCORES)

    dse = nc.dram_tensor("d_sparse", [RPC, W], F32, kind="ExternalInput").ap()
    dde = nc.dram_tensor("d_dense", [RPC, W], F32, kind="ExternalInput").ap()
    mke = nc.dram_tensor("mask", [RPC, W], F32, kind="ExternalInput").ap()
    cse = nc.dram_tensor("cscale", [P, 64], F32, kind="ExternalInput").ap()
    rse = nc.dram_tensor("rscale", [P, 2 * NT], F32,
                     kind="ExternalInput").ap()
    oute = nc.dram_tensor("out", [RPC, W], F32, kind="ExternalOutput").ap()
    if DEBUG:
        dbge = nc.dram_tensor("dbg", [1, 32], F32,
                              kind="ExternalOutput").ap()

    with tile.TileContext(nc) as tc:
        with tc.tile_pool(name="big", bufs=1) as bp, \
             tc.tile_pool(name="small", bufs=1) as sp, \
             tc.tile_pool(name="dram", bufs=1, space="DRAM") as dp:

            mk = [bp.tile([P, W], F32, tag=f"mk{t}", name=f"mk{t}")
                  for t in range(NT)]
            neg = [bp.tile([P, W], F32, tag=f"neg{t}", name=f"neg{t}")
                   for t in range(NT)]
            ds = [bp.tile([P, W], F32, tag=f"ds{t}", name=f"ds{t}")
                  for t in range(NT)]
            dd = [bp.tile([P, W], F32, tag=f"dd{t}", name=f"dd{t}")
                  for t in range(NT)]

            HW2 = W // 2
            for t in range(NT):
                for h in range(2):
                    cs0, cs1 = h * HW2, (h + 1) * HW2
                    nc.sync.dma_start(out=mk[t][:, cs0:cs1],
                                      in_=mke[t * P:(t + 1) * P, cs0:cs1])
                    # neg = (m * BIG) - BIG -> 0 at masked, -BIG at unmasked
                    # (ScalarE: out = Copy(in*scale + bias); keeps DVE free)
                    nc.scalar.activation(neg[t][:, cs0:cs1],
                                         mk[t][:, cs0:cs1], Act.Copy,
                                         bias=-BIG, scale=BIG)
                    nc.sync.dma_start(out=ds[t][:, cs0:cs1],
                                      in_=dse[t * P:(t + 1) * P, cs0:cs1])
            for t in range(NT):
                for h in range(2):
                    cs0, cs1 = h * HW2, (h + 1) * HW2
                    nc.sync.dma_start(out=dd[t][:, cs0:cs1],
                                      in_=dde[t * P:(t + 1) * P, cs0:cs1])

            csc = sp.tile([P, 64], F32, tag="csc", name="csc")
            rsc = sp.tile([P, 2 * NT], F32, tag="rsc", name="rsc")
            # gpsimd (SWDGE) queue: the dynamic-HW DMA queue's completion
            # semaphore is miscounted for these small strided copies on this
            # runtime, letting consumers race ahead of the data.
            nc.gpsimd.dma_start(out=csc[:], in_=cse[:, :])
            nc.gpsimd.dma_start(out=rsc[:], in_=rse[:, :])


            def bounds(dtiles, nm):
                """Compute masked bbox partials of tensor `dtiles`, reduce
                across partitions, and kick off this tensor's AllGather.
                Returns the [8, 6] gathered stats dram tile."""
                ppack = sp.tile([P, 6], F32, tag=f"ppack{nm}",
                                name=f"ppack{nm}")
                nc.vector.memset(ppack[:, 0:3], NEGINF)
                nc.vector.memset(ppack[:, 3:6], POSINF)
                hi = [bp.tile([P, W], F32, tag=f"hi{nm}{t}", name=f"hi{nm}{t}")
                      for t in range(NT)]
                HW2 = W // 2
                rowhi = sp.tile([P, 2 * NT], F32, tag=f"rowhi{nm}",
                                name=f"rowhi{nm}")
                rowlo = sp.tile([P, 2 * NT], F32, tag=f"rowlo{nm}",
                                name=f"rowlo{nm}")
                for t in range(NT):
                    for h in range(2):
                        cs0, cs1 = h * HW2, (h + 1) * HW2
                        k = 2 * t + h
                        # hi = d + neg (d at masked, -BIG unmasked); row max
                        nc.vector._custom_dve(
                            hi_op, out=hi[t][:, cs0:cs1],
                            in0=dtiles[t][:, cs0:cs1],
                            in1=neg[t][:, cs0:cs1],
                            s0=NEGINF, accum_out=rowhi[:, k:k + 1])
                        # depth is non-negative, so the masked-min side is
                        # just min(|hi|): unmasked -BIG -> +BIG, masked d -> d.
                        # Single-source reduce (2x-eligible), no lo tiles.
                        nc.vector.tensor_reduce(
                            rowlo[:, k:k + 1], hi[t][:, cs0:cs1],
                            AxX, Alu.min, apply_absolute_value=True)
                # combine the two row tiles
                nc.vector.tensor_tensor(hi[0][:], hi[0][:], hi[1][:], Alu.max)

                # per-column extremes: fused 32x32 block transpose + in-block
                # row reduce in one TensorReduce(apply_transpose) pass; the
                # min side reads |hi| of the same combined tile.
                colhi = sp.tile([P, 64], F32, tag=f"colhi{nm}", name=f"colhi{nm}")
                collo = sp.tile([P, 64], F32, tag=f"collo{nm}", name=f"collo{nm}")
                nc.vector.tensor_reduce(
                    colhi[:], hi[0][:].rearrange("p (a b) -> p a b", b=32),
                    AxX, Alu.max, apply_transpose=True)
                nc.vector.tensor_reduce(
                    collo[:], hi[0][:].rearrange("p (a b) -> p a b", b=32),
                    AxX, Alu.min, apply_absolute_value=True,
                    apply_transpose=True)
                # colhi[32*pg+a, fg] = extreme of column 32*fg+a over the
                # pg-th 32-row group (of rows folded mod 128 already).  The
                # partition-group fold happens for free in the final
                # partition_all_reduce, so finalize per group directly with
                # cscale replicated per group.

                # x bounds: candidates cscale * colext (both extremes; sign)
                pa = sp.tile([P, 64], F32, tag=f"pa{nm}", name=f"pa{nm}")
                pb = sp.tile([P, 64], F32, tag=f"pb{nm}", name=f"pb{nm}")
                ph = sp.tile([P, 64], F32, tag=f"ph{nm}", name=f"ph{nm}")
                pl = sp.tile([P, 64], F32, tag=f"pl{nm}", name=f"pl{nm}")
                nc.vector.tensor_tensor(pa[:], csc[:], colhi[:], Alu.mult)
                nc.vector.tensor_tensor(pb[:], csc[:], collo[:], Alu.mult)
                nc.vector.tensor_tensor(ph[:], pa[:], pb[:], Alu.max)
                nc.vector.tensor_tensor(pl[:], pa[:], pb[:], Alu.min)
                # (group, column) cells with no masked pixel hold the raw
                # sentinel (colhi == -BIG); their candidates are garbage, so
                # push them to -+3e38 where the max/min reduce ignores them.
                pen = sp.tile([P, 64], F32, tag=f"pen{nm}", name=f"pen{nm}")
                nc.vector.tensor_scalar(pen[:], colhi[:], -BIG, NEGINF,
                                        Alu.is_equal, Alu.mult)
                nc.vector.tensor_tensor(ph[:], ph[:], pen[:], Alu.add)
                nc.vector.tensor_tensor(pl[:], pl[:], pen[:], Alu.subtract)
                nc.vector.tensor_reduce(ppack[:, 0:1], ph[:], AxX, Alu.max)
                nc.vector.tensor_reduce(ppack[:, 3:4], pl[:], AxX, Alu.min)

                # y bounds: candidates rscale * rowext
                qa = sp.tile([P, 2 * NT], F32, tag=f"qa{nm}", name=f"qa{nm}")
                qb = sp.tile([P, 2 * NT], F32, tag=f"qb{nm}", name=f"qb{nm}")
                qh = sp.tile([P, 2 * NT], F32, tag=f"qh{nm}", name=f"qh{nm}")
                ql = sp.tile([P, 2 * NT], F32, tag=f"ql{nm}", name=f"ql{nm}")
                nc.vector.tensor_tensor(qa[:], rsc[:], rowhi[:], Alu.mult)
                nc.vector.tensor_tensor(qb[:], rsc[:], rowlo[:], Alu.mult)
                nc.vector.tensor_tensor(qh[:], qa[:], qb[:], Alu.max)
                nc.vector.tensor_tensor(ql[:], qa[:], qb[:], Alu.min)
                # same sentinel-proofing for fully-unmasked rows
                rpen = sp.tile([P, 2 * NT], F32, tag=f"rpen{nm}", name=f"rpen{nm}")
                nc.vector.tensor_scalar(rpen[:], rowhi[:], -BIG, NEGINF,
                                        Alu.is_equal, Alu.mult)
                nc.vector.tensor_tensor(qh[:], qh[:], rpen[:], Alu.add)
                nc.vector.tensor_tensor(ql[:], ql[:], rpen[:], Alu.subtract)
                nc.vector.tensor_reduce(ppack[:, 1:2], qh[:], AxX, Alu.max)
                nc.vector.tensor_reduce(ppack[:, 4:5], ql[:], AxX, Alu.min)

                # z bounds: plain masked extremes of d
                nc.vector.tensor_reduce(ppack[:, 2:3], rowhi[:],
                                        AxX, Alu.max)
                nc.vector.tensor_reduce(ppack[:, 5:6], rowlo[:],
                                        AxX, Alu.min)

                # negate mins so one max-allreduce covers everything, fold
                # partitions, and AllGather this tensor's 6 stats now --
                # the sparse collective's cross-core skew hides under the
                # dense tensor's compute.
                nc.vector.tensor_scalar(ppack[:, 3:6], ppack[:, 3:6], -1.0,
                                        None, Alu.mult)
                rall = sp.tile([P, 6], F32, tag=f"rall{nm}", name=f"rall{nm}")
                nc.gpsimd.partition_all_reduce(rall[:], ppack[:], channels=P,
                                               reduce_op=bass_isa.ReduceOp.max)
                cc_in = dp.tile([1, 6], F32, tag=f"cc_in{nm}",
                                name=f"cc_in{nm}")
                cc_out = dp.tile([NCORES, 6], F32, tag=f"cc_out{nm}",
                                 name=f"cc_out{nm}", addr_space="Shared")
                nc.gpsimd.dma_start(out=cc_in[:, :], in_=rall[0:1, :])
                nc.gpsimd.collective_compute(
                    "AllGather", Alu.bypass,
                    replica_groups=[list(range(NCORES))],
                    ins=[cc_in.opt()], outs=[cc_out.opt()])
                return cc_out

            cc_out_s = bounds(ds, "s")
            cc_out_d = bounds(dd, "d")

            # gather both tensors' stats and reduce across cores
            g1 = sp.tile([1, 2 * NCORES * 6], F32, tag="g1", name="g1")
            nc.gpsimd.dma_start(out=g1[0:1, 0:48], in_=cc_out_s[:, :])
            nc.gpsimd.dma_start(out=g1[0:1, 48:96], in_=cc_out_d[:, :])
            # gm layout: [smax xyz, snegmin xyz, dmax xyz, dnegmin xyz]
            gm = sp.tile([1, 12], F32, tag="gm", name="gm")
            nc.vector.tensor_reduce(
                gm[0:1, 0:6],
                g1[0:1, 0:48].rearrange("p (c s) -> p s c", c=NCORES),
                AxX, Alu.max)
            nc.vector.tensor_reduce(
                gm[0:1, 6:12],
                g1[0:1, 48:96].rearrange("p (c s) -> p s c", c=NCORES),
                AxX, Alu.max)

            # ratio = ||bbox_s|| / ||bbox_d||; diff = max + (-min), packed
            # [s xyz, d xyz] via one 3D-strided add
            diff = sp.tile([1, 6], F32, tag="diff", name="diff")
            gm3 = gm[0:1, :].rearrange("p (t h c) -> p t h c", t=2, h=2)
            nc.vector.tensor_tensor(
                diff[0:1, :].rearrange("p (t c) -> p t c", t=2),
                gm3[:, :, 0, :], gm3[:, :, 1, :], Alu.add)
            sq = sp.tile([1, 6], F32, tag="sq", name="sq")
            nc.vector.tensor_tensor(sq[0:1, :], diff[0:1, :], diff[0:1, :],
                                    Alu.mult)
            ssum = sp.tile([1, 2], F32, tag="ssum", name="ssum")
            nc.vector.tensor_reduce(
                ssum[0:1, :], sq[0:1, :].rearrange("p (a b) -> p a b", b=3),
                AxX, Alu.add)
            # sqrt via ScalarE LUT (~5e-4 rel) + one Newton step (~1e-7):
            # y1 = 0.5 * (y + x/y)
            sqr = sp.tile([1, 2], F32, tag="sqr", name="sqr")
            zbias = sp.tile([1, 1], F32, tag="zbias", name="zbias")
            nc.vector.memset(zbias[0:1, :], 0.0)
            nc.scalar.activation(sqr[0:1, :], ssum[0:1, :], Act.Sqrt,
                                 bias=zbias[0:1, 0:1])
            yrec = sp.tile([1, 2], F32, tag="yrec", name="yrec")
            nc.vector.reciprocal(yrec[0:1, :], sqr[0:1, :])
            xy = sp.tile([1, 2], F32, tag="xy", name="xy")
            nc.vector.tensor_tensor(xy[0:1, :], ssum[0:1, :], yrec[0:1, :],
                                    Alu.mult)
            nc.vector.tensor_tensor(xy[0:1, :], xy[0:1, :], sqr[0:1, :],
                                    Alu.add)
            nc.vector.tensor_scalar(xy[0:1, :], xy[0:1, :], 0.5, None,
                                    Alu.mult)
            rec = sp.tile([1, 1], F32, tag="rec", name="rec")
            nc.vector.reciprocal(rec[0:1, :], xy[0:1, 1:2])
            ratio = sp.tile([1, 1], F32, tag="ratio", name="ratio")
            nc.vector.tensor_tensor(ratio[0:1, :], xy[0:1, 0:1], rec[0:1, :],
                                    Alu.mult)
            rb = sp.tile([P, 1], F32, tag="rb", name="rb")
            nc.gpsimd.partition_broadcast(rb[:], ratio[0:1, :])

            if DEBUG:
                dbg = sp.tile([1, 32], F32, tag="dbg", name="dbg")
                nc.vector.memset(dbg[0:1, :], 0.0)
                nc.vector.tensor_copy(dbg[0:1, 0:12], gm[0:1, :])
                nc.vector.tensor_copy(dbg[0:1, 12:14], ssum[0:1, :])
                nc.vector.tensor_copy(dbg[0:1, 14:15], ratio[0:1, :])
                nc.sync.dma_start(out=dbge[:, :], in_=dbg[0:1, :])

            # scale the dense shard and store (one tile on ScalarE, one on
            # DVE -- both engines are idle in the tail)
            nc.scalar.mul(dd[0][:], dd[0][:], mul=rb[:, 0:1])
            nc.sync.dma_start(out=oute[0:P, :], in_=dd[0][:])
            nc.vector.tensor_scalar(dd[1][:], dd[1][:], rb[:, 0:1], None,
                                    Alu.mult)
            nc.sync.dma_start(out=oute[P:2 * P, :], in_=dd[1][:])

    nc.compile()
    return nc


LAST_RESULTS = None


def kernel(depth_estimations, sparse_depths, sparse_depth_masks, intrinsics):
    global LAST_RESULTS
    d = np.ascontiguousarray(
        np.asarray(depth_estimations, dtype=np.float32).reshape(H, W))
    s = np.ascontiguousarray(
        np.asarray(sparse_depths, dtype=np.float32).reshape(H, W))
    m = (np.asarray(sparse_depth_masks).reshape(H, W) != 0).astype(np.float32)
    fx, fy, cx, cy = [np.float32(v) for v in np.asarray(intrinsics).ravel()]

    # column scale (x-cx)/fx in transposed-block layout, replicated per
    # partition group: cscT[32*pg + a, fg] = c[32*fg + a]
    c = (np.arange(W, dtype=np.float32) - cx) / fx
    cscT = np.ascontiguousarray(
        np.tile(c.reshape(W // 32, 32).T, (4, 1)))  # [128, 64]

    if "nc" not in _cache:
        _cache["nc"] = _build_nc()
    nc = _cache["nc"]

    in_maps = []
    for i in range(NCORES):
        r0 = i * RPC
        r = (np.arange(r0, r0 + RPC, dtype=np.float32) - cy) / fy
        # [128, 2*NT]: column 2t+h holds row 128t+p's scale (h = tile half)
        rscT = np.ascontiguousarray(
            np.repeat(r.reshape(NT, P).T, 2, axis=1))
        in_maps.append({
            "d_sparse": np.ascontiguousarray(s[r0:r0 + RPC]),
            "d_dense": np.ascontiguousarray(d[r0:r0 + RPC]),
            "mask": np.ascontiguousarray(m[r0:r0 + RPC]),
            "cscale": cscT,
            "rscale": rscT,
        })

    res = run_bass_kernel_spmd(nc, in_maps, list(range(NCORES)),
                               trace=bool(os.environ.get("BASS_TRACE")))
    LAST_RESULTS = res
    out = np.concatenate([res.results[i]["out"] for i in range(NCORES)],
                         axis=0).reshape(1, 1, H, W)
    std = np.float32(1.0)
    return out, std


# revision 35
# speedup vs baseline: 1.6566x; 1.1219x over previous
"""Trainium2 Bass kernel: DepthScalingLayer3D (masked-bbox depth rescale).

Reference semantics:
  mask = sparse_depth_masks[0,0] != 0
  pts(d) = [(x-cx)*d/fx, (y-cy)*d/fy, d]  per pixel
  scale(d) = || masked_max(pts) - masked_min(pts) ||
  out = (scale(sparse)/scale(dense)) * depth_estimations, std = 1.0

Key decomposition: x3 = colscale[j]*d and y3 = rowscale[i]*d with scales
constant per column/row, and fp multiplication by a constant is monotone.
So the global masked min/max of x3/y3/z3 all derive from per-column and
per-row masked min/max of d alone.  Per core (256 rows): two fused
clamp+row-reduce passes per depth tensor (tensor_tensor_reduce), a 32x32
stream-transpose + reduce for per-column extremes, tiny finalize ops,
a 12-float AllGather across the 8 cores, then one scale pass over the
dense shard.

Sharding: rows (H) split 8 ways, 256 rows/core.  kernel() takes the full
inputs, shards on host, runs the SPMD NEFF on cores 0-7, gathers shards.
"""

import os
import sys

import numpy as np

sys.path.insert(0, "/opt/trn_rl_repo")

from concourse import bacc, mybir, tile  # noqa: E402
import concourse.bass_isa as bass_isa  # noqa: E402
from concourse.bass_utils import run_bass_kernel_spmd  # noqa: E402

F32 = mybir.dt.float32
Alu = mybir.AluOpType
Act = mybir.ActivationFunctionType
AxX = mybir.AxisListType.X

P = 128          # SBUF partitions
W = 2048         # image width
H = 2048         # image height
NCORES = 8
RPC = H // NCORES  # rows per core (256)
NT = RPC // P      # row tiles per core (2)
BIG = 1.0e18       # mask sentinel; |d| < 64 so d +- BIG rounds to +-BIG exactly
NEGINF = -3.0e38
POSINF = 3.0e38

# stat columns in the packed [128, 12] partials tile:
#   0..5  = maxes  (sparse x, y, z, dense x, y, z)
#   6..11 = mins (stored negated after finalize) in the same order
_cache = {}
DEBUG = bool(os.environ.get("KERNEL_DEBUG"))


def _register_clamp_ops():
    """Fused (d +- neg) clamp with max/min free-axis accumulate, as custom
    DVE ops (the native TENSOR_TENSOR_REDUCE opcode faults on this runtime).
    Returns (hi_op, lo_op)."""
    from concourse import dve_ops
    from concourse.dve_spec import Spec, Src0, Src1, C0, maxx, minn, lower
    from concourse.dve_spec import _has_src1 as has_src1
    from concourse.dve_uop import DveOpSpec

    existing = {op.name: op for op in dve_ops.OPS}
    if "ANT_CLAMP_HI_MAX" in existing:
        return existing["ANT_CLAMP_HI_MAX"], existing["ANT_CLAMP_LO_MIN"]

    hi = dve_ops.DveOp("ANT_CLAMP_HI_MAX",
                       Spec(body=Src0 + Src1, accum=maxx, accum_init=C0),
                       subdim=False, uops_sha={})
    lo = dve_ops.DveOp("ANT_CLAMP_LO_MIN",
                       Spec(body=Src0 - Src1, accum=minn, accum_init=C0),
                       subdim=False, uops_sha={})
    row = max(dve_ops._SUB_OPCODE_FOR_NAME.values()) + 1
    for op in (hi, lo):
        dve_ops.OPS.append(op)
        dve_ops.CUSTOM_DVE_SPECS[op.name] = op.spec
        dve_ops._SUB_OPCODE_FOR_NAME[op.name] = row
        row += 1
        for ver in ("v3", "v4"):
            try:
                spec = DveOpSpec(
                    name=op.name,
                    opcode=dve_ops.get_dve_sub_opcode(op.name),
                    uops=lower(op.spec, ver=ver),
                    rd1_en=has_src1(op.spec))
                op.uops_sha[ver] = spec.sha(ver)
            except Exception:
                pass
    return hi, lo


def _build_nc():
    hi_op, lo_op = _register_clamp_ops()
    nc = bacc.Bacc("TRN2", target_bir_lowering=False, debug=False,
                   num_devices=NCORES)

    dse = nc.dram_tensor("d_sparse", [RPC, W], F32, kind="ExternalInput").ap()
    dde = nc.dram_tensor("d_dense", [RPC, W], F32, kind="ExternalInput").ap()
    mke = nc.dram_tensor("mask", [RPC, W], F32, kind="ExternalInput").ap()
    cse = nc.dram_tensor("cscale", [P, 64], F32, kind="ExternalInput").ap()
    rse = nc.dram_tensor("rscale", [P, 2 * NT], F32,
                     kind="ExternalInput").ap()
    oute = nc.dram_tensor("out", [RPC, W], F32, kind="ExternalOutput").ap()
    if DEBUG:
        dbge = nc.dram_tensor("dbg", [1, 32], F32,
                              kind="ExternalOutput").ap()

    with tile.TileContext(nc) as tc:
        with tc.tile_pool(name="big", bufs=1) as bp, \
             tc.tile_pool(name="small", bufs=1) as sp, \
             tc.tile_pool(name="dram", bufs=1, space="DRAM") as dp:

            mk = [bp.tile([P, W], F32, tag=f"mk{t}", name=f"mk{t}")
                  for t in range(NT)]
            neg = [bp.tile([P, W], F32, tag=f"neg{t}", name=f"neg{t}")
                   for t in range(NT)]
            ds = [bp.tile([P, W], F32, tag=f"ds{t}", name=f"ds{t}")
                  for t in range(NT)]
            dd = [bp.tile([P, W], F32, tag=f"dd{t}", name=f"dd{t}")
                  for t in range(NT)]

            HW2 = W // 2
            NCH = 2
            WC = W // NCH
            for t in range(NT):
                for h in range(NCH):
                    cs0, cs1 = h * WC, (h + 1) * WC
                    nc.sync.dma_start(out=mk[t][:, cs0:cs1],
                                      in_=mke[t * P:(t + 1) * P, cs0:cs1])
                    # neg = (m * BIG) - BIG -> 0 at masked, -BIG at unmasked
                    # (ScalarE: out = Copy(in*scale + bias); keeps DVE free)
                    nc.scalar.activation(neg[t][:, cs0:cs1],
                                         mk[t][:, cs0:cs1], Act.Copy,
                                         bias=-BIG, scale=BIG)
                    nc.sync.dma_start(out=ds[t][:, cs0:cs1],
                                      in_=dse[t * P:(t + 1) * P, cs0:cs1])
            for t in range(NT):
                for h in range(NCH):
                    cs0, cs1 = h * WC, (h + 1) * WC
                    nc.sync.dma_start(out=dd[t][:, cs0:cs1],
                                      in_=dde[t * P:(t + 1) * P, cs0:cs1])

            csc = sp.tile([P, 64], F32, tag="csc", name="csc")
            rsc = sp.tile([P, 2 * NT], F32, tag="rsc", name="rsc")
            # gpsimd (SWDGE) queue: the dynamic-HW DMA queue's completion
            # semaphore is miscounted for these small strided copies on this
            # runtime, letting consumers race ahead of the data.
            nc.gpsimd.dma_start(out=csc[:], in_=cse[:, :])
            nc.gpsimd.dma_start(out=rsc[:], in_=rse[:, :])


            # shared partials tile: cols 0:3 sparse maxs, 3:6 sparse raw
            # mins, 6:9 dense maxs, 9:12 dense raw mins
            ppk = sp.tile([P, 12], F32, tag="ppk", name="ppk")
            nc.vector.memset(ppk[:, 0:3], NEGINF)
            nc.vector.memset(ppk[:, 3:6], POSINF)
            nc.vector.memset(ppk[:, 6:9], NEGINF)
            nc.vector.memset(ppk[:, 9:12], POSINF)

            def bounds(dtiles, c0, nm):
                """Compute masked bbox partials of tensor `dtiles` into
                ppk[:, c0:c0+6]."""
                ppack = ppk[:, c0:c0 + 6]
                hi = [bp.tile([P, W], F32, tag=f"hi{nm}{t}", name=f"hi{nm}{t}")
                      for t in range(NT)]
                NCH = 2
                WC = W // NCH
                rowhi = sp.tile([P, 2 * NT], F32, tag=f"rowhi{nm}",
                                name=f"rowhi{nm}")
                rowlo = sp.tile([P, 2 * NT], F32, tag=f"rowlo{nm}",
                                name=f"rowlo{nm}")
                for t in range(NT):
                    for h in range(NCH):
                        cs0, cs1 = h * WC, (h + 1) * WC
                        k = NCH * t + h
                        # hi = d + neg (d at masked, -BIG unmasked); row max
                        nc.vector._custom_dve(
                            hi_op, out=hi[t][:, cs0:cs1],
                            in0=dtiles[t][:, cs0:cs1],
                            in1=neg[t][:, cs0:cs1],
                            s0=NEGINF, accum_out=rowhi[:, k:k + 1])
                        # depth is non-negative, so the masked-min side is
                        # just min(|hi|): unmasked -BIG -> +BIG, masked d -> d.
                        # Single-source reduce (2x-eligible), no lo tiles.
                        nc.vector.tensor_reduce(
                            rowlo[:, k:k + 1], hi[t][:, cs0:cs1],
                            AxX, Alu.min, apply_absolute_value=True)
                # combine the two row tiles
                nc.vector.tensor_tensor(hi[0][:], hi[0][:], hi[1][:], Alu.max)

                # per-column extremes: fused 32x32 block transpose + in-block
                # row reduce in one TensorReduce(apply_transpose) pass; the
                # min side reads |hi| of the same combined tile.
                colhi = sp.tile([P, 64], F32, tag=f"colhi{nm}", name=f"colhi{nm}")
                collo = sp.tile([P, 64], F32, tag=f"collo{nm}", name=f"collo{nm}")
                nc.vector.tensor_reduce(
                    colhi[:], hi[0][:].rearrange("p (a b) -> p a b", b=32),
                    AxX, Alu.max, apply_transpose=True)
                nc.vector.tensor_reduce(
                    collo[:], hi[0][:].rearrange("p (a b) -> p a b", b=32),
                    AxX, Alu.min, apply_absolute_value=True,
                    apply_transpose=True)
                # colhi[32*pg+a, fg] = extreme of column 32*fg+a over the
                # pg-th 32-row group (of rows folded mod 128 already).  The
                # partition-group fold happens for free in the final
                # partition_all_reduce, so finalize per group directly with
                # cscale replicated per group.

                # x bounds: candidates cscale * colext (both extremes; sign)
                pa = sp.tile([P, 64], F32, tag=f"pa{nm}", name=f"pa{nm}")
                pb = sp.tile([P, 64], F32, tag=f"pb{nm}", name=f"pb{nm}")
                ph = sp.tile([P, 64], F32, tag=f"ph{nm}", name=f"ph{nm}")
                pl = sp.tile([P, 64], F32, tag=f"pl{nm}", name=f"pl{nm}")
                nc.vector.tensor_tensor(pa[:], csc[:], colhi[:], Alu.mult)
                nc.vector.tensor_tensor(pb[:], csc[:], collo[:], Alu.mult)
                nc.vector.tensor_tensor(ph[:], pa[:], pb[:], Alu.max)
                nc.vector.tensor_tensor(pl[:], pa[:], pb[:], Alu.min)
                # (group, column) cells with no masked pixel hold the raw
                # sentinel (colhi == -BIG); their candidates are garbage, so
                # push them to -+3e38 where the max/min reduce ignores them.
                pen = sp.tile([P, 64], F32, tag=f"pen{nm}", name=f"pen{nm}")
                nc.vector.tensor_scalar(pen[:], colhi[:], -BIG, NEGINF,
                                        Alu.is_equal, Alu.mult)
                nc.vector.tensor_tensor(ph[:], ph[:], pen[:], Alu.add)
                nc.vector.tensor_tensor(pl[:], pl[:], pen[:], Alu.subtract)
                nc.vector.tensor_reduce(ppack[:, 0:1], ph[:], AxX, Alu.max)
                nc.vector.tensor_reduce(ppack[:, 3:4], pl[:], AxX, Alu.min)

                # y bounds: candidates rscale * rowext
                qa = sp.tile([P, 2 * NT], F32, tag=f"qa{nm}", name=f"qa{nm}")
                qb = sp.tile([P, 2 * NT], F32, tag=f"qb{nm}", name=f"qb{nm}")
                qh = sp.tile([P, 2 * NT], F32, tag=f"qh{nm}", name=f"qh{nm}")
                ql = sp.tile([P, 2 * NT], F32, tag=f"ql{nm}", name=f"ql{nm}")
                nc.vector.tensor_tensor(qa[:], rsc[:], rowhi[:], Alu.mult)
                nc.vector.tensor_tensor(qb[:], rsc[:], rowlo[:], Alu.mult)
                nc.vector.tensor_tensor(qh[:], qa[:], qb[:], Alu.max)
                nc.vector.tensor_tensor(ql[:], qa[:], qb[:], Alu.min)
                # same sentinel-proofing for fully-unmasked rows
                rpen = sp.tile([P, 2 * NT], F32, tag=f"rpen{nm}", name=f"rpen{nm}")
                nc.vector.tensor_scalar(rpen[:], rowhi[:], -BIG, NEGINF,
                                        Alu.is_equal, Alu.mult)
                nc.vector.tensor_tensor(qh[:], qh[:], rpen[:], Alu.add)
                nc.vector.tensor_tensor(ql[:], ql[:], rpen[:], Alu.subtract)
                nc.vector.tensor_reduce(ppack[:, 1:2], qh[:], AxX, Alu.max)
                nc.vector.tensor_reduce(ppack[:, 4:5], ql[:], AxX, Alu.min)

                # z bounds: plain masked extremes of d
                nc.vector.tensor_reduce(ppack[:, 2:3], rowhi[:],
                                        AxX, Alu.max)
                nc.vector.tensor_reduce(ppack[:, 5:6], rowlo[:],
                                        AxX, Alu.min)

            bounds(ds, 0, "s")
            bounds(dd, 6, "d")

            # negate mins (strided 3D AP covers both tensors' min cols),
            # fold partitions, AllGather the 12 stats across the chip
            nc.vector.tensor_scalar(
                ppk[:, :].rearrange("p (t h c) -> p t h c", t=2, h=2)
                [:, :, 1, :],
                ppk[:, :].rearrange("p (t h c) -> p t h c", t=2, h=2)
                [:, :, 1, :],
                -1.0, None, Alu.mult)
            rall = sp.tile([P, 12], F32, tag="rall", name="rall")
            nc.gpsimd.partition_all_reduce(rall[:], ppk[:], channels=P,
                                           reduce_op=bass_isa.ReduceOp.max)
            cc_in = dp.tile([1, 12], F32, tag="cc_in", name="cc_in")
            cc_out = dp.tile([NCORES, 12], F32, tag="cc_out", name="cc_out",
                             addr_space="Shared")
            nc.gpsimd.dma_start(out=cc_in[:, :], in_=rall[0:1, :])
            nc.gpsimd.collective_compute(
                "AllGather", Alu.bypass,
                replica_groups=[list(range(NCORES))],
                ins=[cc_in.opt()], outs=[cc_out.opt()])

            # gather and reduce across cores in one strided reduce
            g1 = sp.tile([1, NCORES * 12], F32, tag="g1", name="g1")
            nc.gpsimd.dma_start(out=g1[0:1, :], in_=cc_out[:, :])
            # gm layout: [smax xyz, snegmin xyz, dmax xyz, dnegmin xyz]
            gm = sp.tile([1, 12], F32, tag="gm", name="gm")
            nc.vector.tensor_reduce(
                gm[0:1, :],
                g1[0:1, :].rearrange("p (c s) -> p s c", c=NCORES),
                AxX, Alu.max)

            # ratio = ||bbox_s|| / ||bbox_d||; diff = max + (-min), packed
            # [s xyz, d xyz] via one 3D-strided add
            diff = sp.tile([1, 6], F32, tag="diff", name="diff")
            gm3 = gm[0:1, :].rearrange("p (t h c) -> p t h c", t=2, h=2)
            nc.vector.tensor_tensor(
                diff[0:1, :].rearrange("p (t c) -> p t c", t=2),
                gm3[:, :, 0, :], gm3[:, :, 1, :], Alu.add)
            sq = sp.tile([1, 6], F32, tag="sq", name="sq")
            nc.vector.tensor_tensor(sq[0:1, :], diff[0:1, :], diff[0:1, :],
                                    Alu.mult)
            ssum = sp.tile([1, 2], F32, tag="ssum", name="ssum")
            nc.vector.tensor_reduce(
                ssum[0:1, :], sq[0:1, :].rearrange("p (a b) -> p a b", b=3),
                AxX, Alu.add)
            # sqrt via ScalarE LUT (~5e-4 rel) + one Newton step (~1e-7):
            # y1 = 0.5 * (y + x/y)
            sqr = sp.tile([1, 2], F32, tag="sqr", name="sqr")
            zbias = sp.tile([1, 1], F32, tag="zbias", name="zbias")
            nc.vector.memset(zbias[0:1, :], 0.0)
            nc.scalar.activation(sqr[0:1, :], ssum[0:1, :], Act.Sqrt,
                                 bias=zbias[0:1, 0:1])
            yrec = sp.tile([1, 2], F32, tag="yrec", name="yrec")
            nc.vector.reciprocal(yrec[0:1, :], sqr[0:1, :])
            xy = sp.tile([1, 2], F32, tag="xy", name="xy")
            nc.vector.tensor_tensor(xy[0:1, :], ssum[0:1, :], yrec[0:1, :],
                                    Alu.mult)
            nc.vector.tensor_tensor(xy[0:1, :], xy[0:1, :], sqr[0:1, :],
                                    Alu.add)
            nc.vector.tensor_scalar(xy[0:1, :], xy[0:1, :], 0.5, None,
                                    Alu.mult)
            rec = sp.tile([1, 1], F32, tag="rec", name="rec")
            nc.vector.reciprocal(rec[0:1, :], xy[0:1, 1:2])
            ratio = sp.tile([1, 1], F32, tag="ratio", name="ratio")
            nc.vector.tensor_tensor(ratio[0:1, :], xy[0:1, 0:1], rec[0:1, :],
                                    Alu.mult)
            rb = sp.tile([P, 1], F32, tag="rb", name="rb")
            nc.gpsimd.partition_broadcast(rb[:], ratio[0:1, :])

            if DEBUG:
                dbg = sp.tile([1, 32], F32, tag="dbg", name="dbg")
                nc.vector.memset(dbg[0:1, :], 0.0)
                nc.vector.tensor_copy(dbg[0:1, 0:12], gm[0:1, :])
                nc.vector.tensor_copy(dbg[0:1, 12:14], ssum[0:1, :])
                nc.vector.tensor_copy(dbg[0:1, 14:15], ratio[0:1, :])
                nc.sync.dma_start(out=dbge[:, :], in_=dbg[0:1, :])

            # scale the dense shard and store in half-tile chunks, DVE and
            # ScalarE alternating (both idle in the tail), each chunk's DMA
            # overlapping the next chunk's multiply
            for t in range(NT):
                for h in range(2):
                    cs0, cs1 = h * HW2, (h + 1) * HW2
                    if (2 * t + h) % 2 == 0:
                        nc.vector.tensor_scalar(dd[t][:, cs0:cs1],
                                                dd[t][:, cs0:cs1],
                                                rb[:, 0:1], None, Alu.mult)
                    else:
                        nc.scalar.mul(dd[t][:, cs0:cs1], dd[t][:, cs0:cs1],
                                      mul=rb[:, 0:1])
                    nc.sync.dma_start(out=oute[t * P:(t + 1) * P, cs0:cs1],
                                      in_=dd[t][:, cs0:cs1])

    nc.compile()
    return nc


LAST_RESULTS = None


def kernel(depth_estimations, sparse_depths, sparse_depth_masks, intrinsics):
    global LAST_RESULTS
    d = np.ascontiguousarray(
        np.asarray(depth_estimations, dtype=np.float32).reshape(H, W))
    s = np.ascontiguousarray(
        np.asarray(sparse_depths, dtype=np.float32).reshape(H, W))
    m = (np.asarray(sparse_depth_masks).reshape(H, W) != 0).astype(np.float32)
    fx, fy, cx, cy = [np.float32(v) for v in np.asarray(intrinsics).ravel()]

    # column scale (x-cx)/fx in transposed-block layout, replicated per
    # partition group: cscT[32*pg + a, fg] = c[32*fg + a]
    c = (np.arange(W, dtype=np.float32) - cx) / fx
    cscT = np.ascontiguousarray(
        np.tile(c.reshape(W // 32, 32).T, (4, 1)))  # [128, 64]

    if "nc" not in _cache:
        _cache["nc"] = _build_nc()
    nc = _cache["nc"]

    in_maps = []
    for i in range(NCORES):
        r0 = i * RPC
        r = (np.arange(r0, r0 + RPC, dtype=np.float32) - cy) / fy
        # [128, 2*NT]: column 2t+h holds row 128t+p's scale (h = chunk)
        rscT = np.ascontiguousarray(
            np.repeat(r.reshape(NT, P).T, 2, axis=1))
        in_maps.append({
            "d_sparse": np.ascontiguousarray(s[r0:r0 + RPC]),
            "d_dense": np.ascontiguousarray(d[r0:r0 + RPC]),
            "mask": np.ascontiguousarray(m[r0:r0 + RPC]),
            "cscale": cscT,
            "rscale": rscT,
        })

    res = run_bass_kernel_spmd(nc, in_maps, list(range(NCORES)),
                               trace=bool(os.environ.get("BASS_TRACE")))
    LAST_RESULTS = res
    out = np.concatenate([res.results[i]["out"] for i in range(NCORES)],
                         axis=0).reshape(1, 1, H, W)
    std = np.float32(1.0)
    return out, std
